# revision 34
# baseline (speedup 1.0000x reference)
"""Trainium2 Bass kernel for a full transformer block (LN -> causal MHA -> residual
-> LN -> 4x MLP -> residual), SPMD across 8 NeuronCores.

Sharding: data-parallel over batch (4) x 2-way INTERLEAVED split of query rows.
Core c handles batch b=c//2, parity p=c%2, owning the 128-row q-tiles at global
tile indices {2j+p : j=0..7}. The context shipped to each core is the full 2048
tokens of its batch, ROTATED by -128*p rows, so every core's own q-tile j sits
at context rows [256j, 256j+128) -- the program is SPMD-uniform. For own q-tile
j the causal context is context blocks {0..2j} plus block 15 (which after
rotation is global block 0 for odd cores / a future block for even cores); the
block-15 tile and the diagonal tile are masked multiplicatively post-exp with a
per-core {0,1} bf16 mask on the (otherwise idle) Pool engine. This makes the
causal attention work perfectly balanced across cores with zero pad waste.

Attention uses a transposed PV matmul (lhsT = p [kpos, q]) producing y directly
token-major at full PE rate (65-wide accumulations, contraction=128), with the
softmax denominator as a ones-column of V.

Schedule notes:
- hT is held as 4 chunk tiles (512 ctx cols each) and h2T as 2, so consumers
  wake at chunk granularity instead of waiting for the whole tensor (the Tile
  framework tracks dependencies per tile, not per range).
- Phase A (LN1) emits in groups of 4 context tiles, interleaved with pair 0's
  K/V projection chunks in input-availability order, so the in-order PE queue
  always has ready work while the LN pipeline streams; group-0 x tiles are
  issued ahead of any weight DMA.
- Each pair's QKV weights are prefetched during the previous pair's attention;
  the first MLP weight slice and the first phase-D residual x tiles transfer
  during the BC tail so the BC->D boundary isn't DMA-congested.
- Elementwise work is spread across engines: LN applies + transpose copybacks
  on Act, attention masking + the x2+b2 stash on Pool, the rest on DVE. The
  attention-output bias bo is added on DVE (replacing a PE bias matmul).
- The two heads' K=64 score matmuls are interleaved block-by-block so they
  land on disjoint PE row groups ((0,0) / (64,0) via base_partition
  auto-derive) and execute concurrently in the array, recovering most of the
  half-rate penalty of the D=64 contraction.

Matmul operands are bf16 (fp8 fails the accuracy budget; verified empirically:
even a single fp8-e4m3 matmul site pushes max-rel-err past the 2e-2 gate).
All accumulation/LN/softmax math is fp32. Weights arrive host-prepacked in SBUF
layout so every weight DMA is contiguous per partition.
"""
import contextlib
from types import SimpleNamespace

import ml_dtypes
import numpy as np

import concourse.bass as bass
import concourse.bacc as bacc
import concourse.tile as tile
import concourse.mybir as mybir
from concourse.masks import make_identity

F32 = mybir.dt.float32
BF16 = mybir.dt.bfloat16
AF = mybir.ActivationFunctionType
ALU = mybir.AluOpType
P = 128

MM_DT = BF16


def _np_mm_dt(mm_dt):
    return ml_dtypes.bfloat16 if mm_dt == BF16 else np.float32


def _bcast_ap(ap, parts=P):
    """[N] dram AP -> [parts, N] broadcast AP (step-0 partition dim)."""
    return bass.AP(tensor=ap.tensor, offset=ap.offset, ap=[[0, parts]] + list(ap.ap))


def _rep_free_ap(ap, n):
    """[P, N] sbuf AP -> [P, n, N] with a 0-stride middle free dim."""
    a = list(ap.ap)
    return bass.AP(tensor=ap.tensor, offset=ap.offset,
                   ap=[a[0], [0, n]] + a[1:])


def _ln_tile(nc, E, pool_stats, src, dst, eps_t, tag):
    """LayerNorm one [P, C] tile: dst = (src - mu) * rstd on the Act engine.
    The per-feature w/b are applied later (folded into weights/biases)."""
    C = src.shape[-1]
    nsg = max(1, C // 512)
    sg_sz = min(512, C)
    st = pool_stats.tile([P, nsg * 6 + 4], F32, tag=f"st{tag}", name="st")
    stats = st[:, 0:nsg * 6].rearrange("p (n s) -> p n s", s=6)
    mv = st[:, nsg * 6:nsg * 6 + 2]
    rstd = st[:, nsg * 6 + 2:nsg * 6 + 3]
    nb = st[:, nsg * 6 + 3:nsg * 6 + 4]
    for sg in range(nsg):
        nc.vector.bn_stats(out=stats[:, sg, :],
                           in_=src[:, sg * sg_sz:(sg + 1) * sg_sz])
    nc.vector.bn_aggr(out=mv, in_=stats)
    nc.scalar.activation(rstd, mv[:, 1:2], AF.Sqrt, bias=eps_t, scale=1.0)
    nc.vector.reciprocal(rstd, rstd)
    # dst = src*rstd - mu*rstd on the Act engine (frees DVE)
    nc.vector.scalar_tensor_tensor(out=nb, in0=mv[:, 0:1], scalar=-1.0,
                                   in1=rstd, op0=ALU.mult, op1=ALU.mult)
    nc.scalar.activation(dst, src, AF.Identity, bias=nb, scale=rstd)


def _transpose_tile(nc, E, G, psum_pool, src, dstT, col0, tag, cb=None):
    """Transpose [P, C] token-major tile into feature-major dstT[:, :, col0:+P],
    batching TG 128x128 transposes per PSUM bank; copyback engine `cb`."""
    CC = src.shape[-1] // P
    TG = min(4, CC)
    for cc0 in range(0, CC, TG):
        pt = psum_pool.tile([P, TG, P], src.dtype, tag=tag, name="pt")
        for j in range(TG):
            nc.tensor.transpose(pt[:, j, :],
                                src[:, (cc0 + j) * P:(cc0 + j + 1) * P], G.ident)
        (cb or nc.scalar.copy)(dstT[:, cc0:cc0 + TG, col0:col0 + P], pt)


def _alloc_pair(nc, E, G, wp, kqgp, vtokp, g):
    """Allocate one head-pair's weight/output tiles and start weight DMAs."""
    S = SimpleNamespace(g=g)
    S.wk_t = wp.tile([P, E.CCH, P], E.MMDT, tag="wk", name="wk_t")
    nc.gpsimd.dma_start(out=S.wk_t, in_=G.Wk4[:, g])
    S.wv_t = wp.tile([P, E.CCH, P], E.MMDT, tag="wv", name="wv_t")
    nc.gpsimd.dma_start(out=S.wv_t, in_=G.Wv4[:, g])
    S.wq_t = wp.tile([P, E.CCH, P], E.MMDT, tag="wq", name="wq_t")
    nc.gpsimd.dma_start(out=S.wq_t, in_=G.Wq4[:, g])
    S.kTg = kqgp.tile([P, E.T], E.MMDT, tag="k", name="kTg")
    S.qTg = kqgp.tile([P, E.TQ], E.MMDT, tag="q", name="qTg")
    S.vTok = vtokp.tile([P, E.NT, 2, E.D + 1], E.MMDT, tag="vt", name="vTok")
    nc.gpsimd.memset(S.vTok[:, :, :, E.D:E.D + 1], 1.0)
    return S


def _emit_kv_chunk(nc, E, G, prjp, S, tcn):
    """K projection for context chunk tcn + V projection for kt group 4*tcn."""
    g = S.g
    kps = prjp.tile([P, E.TC], F32, tag="pp", name="kps")
    for ci in range(E.CCH):
        nc.tensor.matmul(kps, S.wk_t[:, ci, :], G.hTc[tcn][:, ci, :],
                         start=(ci == 0), stop=(ci == E.CCH - 1))
    nc.vector.tensor_scalar_add(
        out=S.kTg[:, tcn * E.TC:(tcn + 1) * E.TC], in0=kps,
        scalar1=G.bk_s[:, g:g + 1])
    kt0 = 4 * tcn
    vps = prjp.tile([P, 4, P], F32, tag="pp", name="vps")
    for kk in range(4):
        for ci in range(E.CCH):
            nc.tensor.matmul(
                vps[:, kk, :], G.hTc[tcn][:, ci, kk * P:(kk + 1) * P],
                S.wv_t[:, ci, :],
                start=(ci == 0), stop=(ci == E.CCH - 1))
    bv_g = _rep_free_ap(
        G.bv_b[:, g * P:(g + 1) * P].rearrange("p (b c) -> p b c", c=E.D), 4)
    nc.vector.tensor_tensor(
        S.vTok[:, kt0:kt0 + 4, :, 0:E.D],
        vps.rearrange("p a (b c) -> p a b c", c=E.D), bv_g, ALU.add)


def _emit_q(nc, E, G, prjp, S):
    """Q projection (feature-major, own interleaved rows, per hT chunk)."""
    g = S.g
    for qc in range(E.TQ // 512):
        qps = prjp.tile([P, 512], F32, tag="pp", name="qps")
        for half in range(2):
            ch = 2 * qc + half
            rhs = G.hTc[ch].rearrange("p c (j u) -> p c j u", u=2 * P)
            for ci in range(E.CCH):
                nc.tensor.matmul(
                    qps[:, half * 256:(half + 1) * 256],
                    S.wq_t[:, ci, :], rhs[:, ci, :, 0:P],
                    start=(ci == 0), stop=(ci == E.CCH - 1))
        nc.vector.tensor_scalar_add(
            out=S.qTg[:, qc * 512:(qc + 1) * 512], in0=qps,
            scalar1=G.bq_s[:, g:g + 1])


def _emit_attn(nc, E, G, attp, recp, ytmp, spsp, ypsp, S):
    """Causal attention for one head-pair over own q-tiles; y -> yT via XBAR."""
    g = S.g
    kTg, qTg, vTok = S.kTg, S.qTg, S.vTok
    y_tm = ytmp.tile([P, E.NQT, P], E.MMDT, tag="ytm", name="y_tm")
    for j in range(E.NQT):
        nkt = 2 * j + 2
        yp = ypsp.tile([P, 2, 256], F32, tag="yp", name="yp")
        # Interleave the two heads' K=64 score matmuls block-by-block: head 0
        # reads kTg/qTg partitions 0-63 (row group (0,0)), head 1 partitions
        # 64-127 ((64,0) via base_partition auto-derive), so adjacent
        # instructions land on disjoint PE row groups and run concurrently.
        p_sbs = [attp.tile([P, 16, P], E.MMDT, tag="p", name="p_sb")
                 for _ in range(2)]
        for c0 in range(0, nkt, 8):
            cn = min(8, nkt - c0)
            sps = [spsp.tile([P, 8, P], F32, tag="sp", name="sp")
                   for _ in range(2)]
            for tt in range(cn):
                t = c0 + tt
                slot = t if t <= 2 * j else 15
                for hh in range(2):
                    poff = hh * E.D
                    nc.tensor.matmul(
                        sps[hh][:, tt, :],
                        kTg[poff:poff + E.D, slot * P:(slot + 1) * P],
                        qTg[poff:poff + E.D, j * P:(j + 1) * P],
                        start=(tt % 4 == 0),
                        stop=(tt % 4 == 3 or tt == cn - 1),
                        skip_group_check=True)
            for hh in range(2):
                nc.scalar.activation(p_sbs[hh][:, c0:c0 + cn, :],
                                     sps[hh][:, 0:cn, :], AF.Exp, scale=E.SD)
        # mask diagonal tile (tri) + block-15 tile (parity 0/1) on Pool
        for hh in range(2):
            nc.gpsimd.tensor_tensor(
                p_sbs[hh][:, 2 * j:2 * j + 2, :],
                p_sbs[hh][:, 2 * j:2 * j + 2, :],
                G.mask2, ALU.mult)
        for hh in range(2):
            for t in range(nkt):
                slot = t if t <= 2 * j else 15
                nc.tensor.matmul(
                    yp[:, hh, 0:E.D + 1], p_sbs[hh][:, t, :],
                    vTok[:, slot, hh, :],
                    start=(hh == 0 and t == 0), stop=(t == nkt - 1),
                    skip_group_check=True)
        rec2 = recp.tile([P, 2], F32, tag="r", name="rec2")
        nc.vector.reciprocal(rec2, yp[:, :, E.D])
        for hh in range(2):
            nc.vector.tensor_scalar_mul(
                y_tm[:, j, hh * E.D:(hh + 1) * E.D],
                yp[:, hh, 0:E.D], scalar1=rec2[:, hh:hh + 1])
    # this pair's y columns -> feature-major yT (PE transpose, DVE copyback)
    for j0 in range(0, E.NQT, 4):
        pt = ypsp.tile([P, 4, P], E.MMDT, tag="yp", name="ypt")
        for jj in range(4):
            nc.tensor.transpose(pt[:, jj, :], y_tm[:, j0 + jj, :], G.ident)
        nc.vector.tensor_copy(G.yT[:, g, j0 * P:(j0 + 4) * P], pt)


def _phase_abc(nc, tc, E, G, xrp):
    """LN1 (grouped, interleaved with pair 0's K/V) + all pairs' QKV + attn.

    DMA choreography: group-0 x tiles are issued before any weight DMA so the
    LN pipeline starts immediately; each pair's weights are prefetched during
    the previous pair's attention; the first MLP weight slice and the first
    phase-D residual x tiles are prefetched during the last pairs' attention
    so the BC->D boundary isn't DMA-congested."""
    with contextlib.ExitStack() as st:
        wp = st.enter_context(tc.tile_pool(name="wkqv", bufs=2))
        kqgp = st.enter_context(tc.tile_pool(name="kqg", bufs=2))
        vtokp = st.enter_context(tc.tile_pool(name="vtok", bufs=2))
        attp = st.enter_context(tc.tile_pool(name="att", bufs=4))
        recp = st.enter_context(tc.tile_pool(name="rec", bufs=3))
        ytmp = st.enter_context(tc.tile_pool(name="ytm", bufs=2))
        prjp = st.enter_context(tc.tile_pool(name="prj", bufs=2, space="PSUM"))

        # --- phase A: LN1 in 4-tile groups; after each group, emit pair 0's
        # AND pair 1's K/V work for the chunk that just completed, so the PE
        # has ~6.8us of projection work per ~6us LN group ---
        with tc.tile_pool(name="lna", bufs=6) as lna, \
             tc.tile_pool(name="lnst", bufs=6) as lnst, \
             tc.tile_pool(name="tpsA", bufs=3, space="PSUM") as tpsA:
            # group-0 x tiles first in the DMA queue
            pre_x = {}
            for rr in range(4):
                rt = 4 * (E.NTC - 1) + rr
                x_t = lna.tile([P, E.C], F32, tag="x", name="x_t")
                nc.sync.dma_start(x_t, G.x[rt * P:(rt + 1) * P, :])
                pre_x[rt] = x_t
            S0 = _alloc_pair(nc, E, G, wp, kqgp, vtokp, 0)
            nc.gpsimd.dma_start(out=G.bv_b, in_=_bcast_ap(G.bv_d))
            for tcn in (E.NTC - 1,) + tuple(range(E.NTC - 1)):
                for rr in range(4):
                    rt = 4 * tcn + rr
                    x_t = pre_x.pop(rt, None)
                    if x_t is None:
                        x_t = lna.tile([P, E.C], F32, tag="x", name="x_t")
                        nc.sync.dma_start(x_t, G.x[rt * P:(rt + 1) * P, :])
                    h_t = lna.tile([P, E.C], E.MMDT, tag="h", name="h_t")
                    _ln_tile(nc, E, lnst, x_t, h_t, G.eps_t, "a")
                    _transpose_tile(nc, E, G, tpsA, h_t, G.hTc[tcn],
                                    rr * P, "tp")
                _emit_kv_chunk(nc, E, G, prjp, S0, tcn)

        _emit_q(nc, E, G, prjp, S0)

        with tc.tile_pool(name="sps", bufs=2, space="PSUM") as spsp, \
             tc.tile_pool(name="yps", bufs=2, space="PSUM") as ypsp:
            S = S0
            for g in range(1, E.NP + 1):
                S_next = (None if g == E.NP else
                          _alloc_pair(nc, E, G, wp, kqgp, vtokp, g))
                if g == E.NP - 2:
                    # phase-D operands (Wo + bias broadcasts), mid-BC
                    nc.gpsimd.dma_start(out=G.wo_t, in_=G.Wo3)
                    nc.gpsimd.dma_start(out=G.bo_b, in_=_bcast_ap(G.bo_d))
                    nc.gpsimd.dma_start(out=G.b2_b, in_=_bcast_ap(G.b2_d))
                if g == E.NP - 1:
                    # first residual x tiles, transferred during the BC tail
                    G.xr_pre = []
                    for j in range(2):
                        xr_t = xrp.tile([P, E.C], F32, tag="xr", name="xr_t")
                        nc.sync.dma_start(
                            out=xr_t, in_=G.x[2 * j * P:(2 * j + 1) * P, :])
                        G.xr_pre.append(xr_t)
                _emit_attn(nc, E, G, attp, recp, ytmp, spsp, ypsp, S)
                if g < E.NP:
                    for tcn in (E.NTC - 1,) + tuple(range(E.NTC - 1)):
                        _emit_kv_chunk(nc, E, G, prjp, S_next, tcn)
                    _emit_q(nc, E, G, prjp, S_next)
                    S = S_next


def _phase_d(nc, tc, E, G, xrp):
    """O-projection + residual + LN2 -> h2T (DMA XBAR); x2+b2 -> x2b_sb."""
    wo_t = G.wo_t
    with tc.tile_pool(name="dwork", bufs=3) as dwork, \
         tc.tile_pool(name="lnst2", bufs=4) as lnst2, \
         tc.tile_pool(name="ops", bufs=4, space="PSUM") as opsp, \
         tc.tile_pool(name="tpsD", bufs=2, space="PSUM") as tpsD:
        for j in range(E.NQT):
            if j < len(G.xr_pre):
                xr_t = G.xr_pre[j]
            else:
                xr_t = xrp.tile([P, E.C], F32, tag="xr", name="xr_t")
                nc.sync.dma_start(out=xr_t,
                                  in_=G.x[2 * j * P:(2 * j + 1) * P, :])
            # xrb = x + bo on Pool (replaces the ones-column bias matmul
            # that used to burn PE cycles)
            xrb_t = dwork.tile([P, E.C], F32, tag="xrb", name="xrb_t")
            nc.vector.tensor_tensor(xrb_t, xr_t, G.bo_b, ALU.add)
            x2_t = dwork.tile([P, E.C], F32, tag="x2", name="x2_t")
            for oc in range(E.NOC):
                ops = opsp.tile([P, E.OC], F32, tag="op", name="ops")
                for ci in range(E.CCH):
                    nc.tensor.matmul(ops, G.yT[:, ci, j * P:(j + 1) * P],
                                     wo_t[:, ci, oc * E.OC:(oc + 1) * E.OC],
                                     start=(ci == 0), stop=(ci == E.CCH - 1),
                                     skip_group_check=True)
                nc.vector.tensor_tensor(
                    x2_t[:, oc * E.OC:(oc + 1) * E.OC], ops,
                    xrb_t[:, oc * E.OC:(oc + 1) * E.OC], ALU.add)
            nc.gpsimd.tensor_tensor(G.x2b_sb[:, j, :], x2_t, G.b2_b, ALU.add)
            h2_t = dwork.tile([P, E.C], E.MMDT, tag="h2", name="h2_t")
            _ln_tile(nc, E, lnst2, x2_t, h2_t, G.eps_t, "d")
            _transpose_tile(nc, E, G, tpsD, h2_t, G.h2Tc[j // 4],
                            (j % 4) * P, "tp")


def _phase_e(nc, tc, E, G, w1p, w2p, w_pre):
    """MLP: u = relu(h2 @ W1 + b1); y_acc = u @ W2, sliced over F."""
    with tc.tile_pool(name="uall", bufs=2) as uallp, \
         tc.tile_pool(name="ups", bufs=3, space="PSUM") as upsp, \
         tc.tile_pool(name="ypsE", bufs=3, space="PSUM") as ypsEp:
        for fs in range(E.NFS):
            if fs == 0:
                w1_t, w2_t = w_pre
            else:
                w1_t = w1p.tile([P, E.CCH, E.FS], E.MMDT, tag="w1", name="w1_t")
                nc.gpsimd.dma_start(out=w1_t, in_=G.W14[:, fs])
                w2_t = w2p.tile([P, E.NFC, E.C], E.MMDT, tag="w2", name="w2_t")
                nc.gpsimd.dma_start(out=w2_t, in_=G.W24[:, fs])
            for tq in range(E.NTQC):
                u_all = uallp.tile([P, E.NFC, E.TQC], E.MMDT, tag="ua",
                                   name="u_all")
                for fc in range(E.NFC):
                    ups = upsp.tile([P, E.TQC], F32, tag="u", name="ups")
                    for ci in range(E.CCH):
                        nc.tensor.matmul(
                            ups, w1_t[:, ci, fc * P:(fc + 1) * P],
                            G.h2Tc[tq][:, ci, :],
                            start=(ci == 0), stop=(ci == E.CCH - 1))
                    fi = fs * E.NFC + fc
                    nc.scalar.activation(u_all[:, fc, :], ups, AF.Relu,
                                         bias=G.b1_s[:, fi:fi + 1], scale=1.0)
                for t2 in range(E.TSUB):
                    tt = tq * E.TSUB + t2
                    for oc in range(E.NOC):
                        yps = ypsEp.tile([P, E.OC], F32, tag="y", name="yps")
                        for fc in range(E.NFC):
                            nc.tensor.matmul(
                                yps, u_all[:, fc, t2 * P:(t2 + 1) * P],
                                w2_t[:, fc, oc * E.OC:(oc + 1) * E.OC],
                                start=(fc == 0), stop=(fc == E.NFC - 1))
                        dst = G.y_acc[:, tt, oc * E.OC:(oc + 1) * E.OC]
                        if fs == 0:
                            nc.vector.tensor_copy(dst, yps)
                        elif fs < E.NFS - 1:
                            nc.vector.tensor_tensor(dst, dst, yps, ALU.add)
                        else:
                            # last slice: fuse accumulate + residual, DMA out
                            o_t = G.finp.tile([P, E.OC], F32, tag="o",
                                              name="o_t")
                            nc.vector.tensor_tensor(o_t, yps, dst, ALU.add)
                            o_t2 = G.finp.tile([P, E.OC], F32, tag="o2",
                                               name="o_t2")
                            eng = nc.gpsimd if (2 * tt + oc) % 2 else nc.vector
                            eng.tensor_tensor(
                                o_t2, o_t,
                                G.x2b_sb[:, tt, oc * E.OC:(oc + 1) * E.OC],
                                ALU.add)
                            nc.sync.dma_start(
                                G.out[tt * P:(tt + 1) * P,
                                      oc * E.OC:(oc + 1) * E.OC], o_t2)


def _trace_main(nc, tc, E, G):
    """One full block computation."""
    with tc.tile_pool(name="h2T", bufs=1) as h2Tp:
        with tc.tile_pool(name="yT", bufs=1) as yTp, \
             tc.tile_pool(name="wo", bufs=1) as wop, \
             tc.tile_pool(name="xrp", bufs=2) as xrp:
            with tc.tile_pool(name="hT", bufs=1) as hTp:
                G.hTc = [hTp.tile([P, E.CCH, E.TC], E.MMDT,
                                  tag=f"h{c}", name=f"hT{c}")
                         for c in range(E.NTC)]
                G.yT = yTp.tile([P, E.CCH, E.TQ], E.MMDT, tag="yT",
                                name="yT")
                G.wo_t = wop.tile([P, E.CCH, E.C], E.MMDT, name="wo_t")
                # wo_t DMA is issued inside _phase_abc (late BC) so its 2MB
                # transfer doesn't compete with the group-0 x tiles at t=0
                _phase_abc(nc, tc, E, G, xrp)
            G.h2Tc = [h2Tp.tile([P, E.CCH, 512], E.MMDT, tag=f"h2{c}",
                                name=f"h2T{c}") for c in range(E.NTQC)]
            G.x2b_sb = h2Tp.tile([P, E.NQT, E.C], E.MMDT, tag="x2b",
                                 name="x2b_sb")
            # w1/w2 pools open after hT frees its space; the first slice's
            # transfers start eagerly during the BC tail / phase D
            with tc.tile_pool(name="w1", bufs=2) as w1p, \
                 tc.tile_pool(name="w2", bufs=2) as w2p:
                w1_t0 = w1p.tile([P, E.CCH, E.FS], E.MMDT, tag="w1",
                                 name="w1_t")
                nc.gpsimd.dma_start(out=w1_t0, in_=G.W14[:, 0])
                w2_t0 = w2p.tile([P, E.NFC, E.C], E.MMDT, tag="w2",
                                 name="w2_t")
                nc.gpsimd.dma_start(out=w2_t0, in_=G.W24[:, 0])
                _phase_d(nc, tc, E, G, xrp)

                with tc.tile_pool(name="yacc", bufs=1) as yaccp, \
                     tc.tile_pool(name="fin", bufs=2) as finp:
                    G.y_acc = yaccp.tile([P, E.NQT, E.C], F32, name="y_acc")
                    G.finp = finp
                    _phase_e(nc, tc, E, G, w1p, w2p, (w1_t0, w2_t0))


def build_nc(T=2048, TQ=1024, C=1024, H=16, D=64, F=4096, n_cores=8,
             mm_dt=MM_DT, body_reps=1):
    E = SimpleNamespace(T=T, TQ=TQ, C=C, H=H, D=D, F=F, MMDT=mm_dt)
    E.CCH = C // P
    E.NT = T // P
    E.NQT = TQ // P
    E.NP = H // 2
    E.TC = min(512, T)
    E.NTC = T // E.TC
    E.OC = min(512, C)
    E.NOC = C // E.OC
    E.FS = min(1024, F)
    E.NFS = F // E.FS
    E.NFC = E.FS // P
    E.TQC = min(512, TQ)
    E.NTQC = TQ // E.TQC
    E.TSUB = E.TQC // P
    E.SD = float(1.0 / np.sqrt(D))

    nc = bacc.Bacc("TRN2", target_bir_lowering=False, debug=False,
                   num_devices=n_cores)
    G = SimpleNamespace()
    G.x = nc.dram_tensor("x", [T, C], F32, kind="ExternalInput").ap()
    G.mask2_d = nc.dram_tensor("mask2", [P, 2 * P], mm_dt,
                               kind="ExternalInput").ap()
    G.Wq4 = nc.dram_tensor("Wq", [P, E.NP, E.CCH, P], mm_dt,
                           kind="ExternalInput").ap()
    G.Wk4 = nc.dram_tensor("Wk", [P, E.NP, E.CCH, P], mm_dt,
                           kind="ExternalInput").ap()
    G.Wv4 = nc.dram_tensor("Wv", [P, E.NP, E.CCH, P], mm_dt,
                           kind="ExternalInput").ap()
    G.Wo3 = nc.dram_tensor("Wo", [P, E.CCH, C], mm_dt,
                           kind="ExternalInput").ap()
    G.W14 = nc.dram_tensor("W1", [P, E.NFS, E.CCH, E.FS], mm_dt,
                           kind="ExternalInput").ap()
    G.W24 = nc.dram_tensor("W2", [P, E.NFS, E.NFC, C], mm_dt,
                           kind="ExternalInput").ap()
    vecs = {}
    for nm in ("bq", "bk", "bv", "bo", "b2"):
        vecs[nm] = nc.dram_tensor(nm, [C], F32, kind="ExternalInput").ap()
    vecs["b1"] = nc.dram_tensor("b1", [F], F32, kind="ExternalInput").ap()
    G.bv_d, G.bo_d, G.b2_d = vecs["bv"], vecs["bo"], vecs["b2"]
    G.out = nc.dram_tensor("out", [TQ, C], F32, kind="ExternalOutput").ap()

    with tile.TileContext(nc) as tc, contextlib.ExitStack() as ctx:
        glob = ctx.enter_context(tc.tile_pool(name="glob", bufs=1))

        G.ident = glob.tile([P, P], E.MMDT, name="ident")
        make_identity(nc, G.ident)
        G.mask2 = glob.tile([P, 2, P], E.MMDT, name="mask2")
        nc.gpsimd.dma_start(out=G.mask2,
                            in_=G.mask2_d.rearrange("p (a b) -> p a b", b=P))

        nsm = 1 + 2 * E.CCH + F // P
        sm = glob.tile([P, nsm], F32, name="sm")
        o = 0
        G.eps_t = sm[:, o:o + 1]; o += 1
        G.bq_s = sm[:, o:o + E.CCH]; o += E.CCH
        G.bk_s = sm[:, o:o + E.CCH]; o += E.CCH
        G.b1_s = sm[:, o:o + F // P]; o += F // P
        nc.vector.memset(G.eps_t, 1e-5)
        nc.sync.dma_start(G.bq_s, vecs["bq"].rearrange("(o p) -> p o", p=P))
        nc.sync.dma_start(G.bk_s, vecs["bk"].rearrange("(o p) -> p o", p=P))
        nc.sync.dma_start(G.b1_s, vecs["b1"].rearrange("(o p) -> p o", p=P))

        # [P, C] broadcast bias tiles; DMAs issued phase-ordered in the body
        G.b2_b = glob.tile([P, E.C], F32, tag="b2_b", name="b2_b")
        G.bv_b = glob.tile([P, E.C], F32, tag="bv_b", name="bv_b")
        G.bo_b = glob.tile([P, E.C], F32, tag="bo_b", name="bo_b")

        for _rep in range(body_reps):
            _trace_main(nc, tc, E, G)

    nc.compile()
    return nc


# ----------------------------------------------------------------------------
# Host entry point: takes FULL inputs, shards, runs 8 cores, gathers.
# ----------------------------------------------------------------------------
_NC_CACHE = {}


def _get_nc():
    if "full" not in _NC_CACHE:
        _NC_CACHE["full"] = build_nc()
    return _NC_CACHE["full"]


def _pack_weights(inputs, T, TQ, C, H, D, F, mm_dt=MM_DT):
    """Prepack weights into contiguous per-partition SBUF layouts."""
    wdt = _np_mm_dt(mm_dt)
    CCH = C // P
    NP = H // 2
    FS = min(1024, F)
    NFS = F // FS
    NFC = FS // P

    ln1w = np.asarray(inputs["ln1_w"], dtype=np.float32)
    ln2w = np.asarray(inputs["ln2_w"], dtype=np.float32)

    def w(k):
        a = np.asarray(inputs[k], dtype=np.float32)
        if k in ("Wq", "Wk", "Wv"):
            a = a * ln1w[:, None]   # fold LN1 scale
        elif k == "W1":
            a = a * ln2w[:, None]   # fold LN2 scale
        return a.astype(wdt)

    out = {}
    # [C_in, M] -> [p, pair, ci, m]
    for k in ("Wq", "Wk", "Wv"):
        out[k] = np.ascontiguousarray(
            w(k).reshape(CCH, P, NP, P).transpose(1, 2, 0, 3))
    out["Wo"] = np.ascontiguousarray(
        w("Wo").reshape(CCH, P, C).transpose(1, 0, 2))
    out["W1"] = np.ascontiguousarray(
        w("W1").reshape(CCH, P, NFS, FS).transpose(1, 2, 0, 3))
    out["W2"] = np.ascontiguousarray(
        w("W2").reshape(NFS, NFC, P, C).transpose(2, 0, 1, 3))
    return out


def prepare_common(inputs, T, TQ, C, H, D, F, mm_dt=MM_DT):
    f32 = lambda k: np.asarray(inputs[k], dtype=np.float32)
    ln1b, ln2b = f32("ln1_b"), f32("ln2_b")
    common = {
        # fold LN1/LN2 bias through the following matmul into its bias
        "bq": np.ascontiguousarray(ln1b @ f32("Wq") + f32("bq")),
        "bk": np.ascontiguousarray(ln1b @ f32("Wk") + f32("bk")),
        "bv": np.ascontiguousarray(ln1b @ f32("Wv") + f32("bv")),
        "b1": np.ascontiguousarray(ln2b @ f32("W1") + f32("b1")),
        "bo": np.ascontiguousarray(f32("bo")),
        "b2": np.ascontiguousarray(f32("b2")),
    }
    common.update(_pack_weights(inputs, T, TQ, C, H, D, F, mm_dt))
    return common


def make_in_maps(inputs, n_cores=8, mm_dt=MM_DT):
    wdt = _np_mm_dt(mm_dt)
    x = np.asarray(inputs["x"], dtype=np.float32)
    B, T, C = x.shape
    TQ = (B * T) // n_cores
    H, D, F = 16, 64, 4096
    common = prepare_common(inputs, T, TQ, C, H, D, F, mm_dt)
    # mask2[:, 0, :]: diagonal tri (kk <= i); mask2[:, 1, :]: parity block-15
    tri01 = (np.arange(P)[:, None] <= np.arange(P)[None, :]).astype(np.float32)
    in_maps = []
    for c in range(n_cores):
        b, p = c // 2, c % 2
        x_ctx = np.ascontiguousarray(np.roll(x[b], -P * p, axis=0))
        m2 = np.empty((P, 2 * P), dtype=wdt)
        m2[:, 0:P] = tri01.astype(wdt)
        m2[:, P:2 * P] = wdt(float(p))
        in_maps.append({"x": x_ctx, "mask2": m2, **common})
    return in_maps


def core_out_rows(c, TQ=1024):
    """Global (batch, row) ranges covered by core c's output rows."""
    b, p = c // 2, c % 2
    return b, [(256 * j + P * p, 128 * j) for j in range(TQ // P)]


def kernel(**inputs):
    from concourse.bass_utils import run_bass_kernel_spmd

    x = np.asarray(inputs["x"], dtype=np.float32)
    B, T, C = x.shape          # (4, 2048, 1024)
    n_cores = 8
    TQ = (B * T) // n_cores    # 1024 query rows per core

    nc = _get_nc()
    in_maps = make_in_maps(inputs, n_cores)
    res = run_bass_kernel_spmd(nc, in_maps, core_ids=list(range(n_cores)),
                               trace=False)

    out = np.empty((B, T, C), dtype=np.float32)
    for c in range(n_cores):
        b, p = c // 2, c % 2
        r = res.results[c]["out"]
        for j in range(TQ // P):
            out[b, 256 * j + P * p:256 * j + P * p + P, :] = \
                r[j * P:(j + 1) * P, :]
    return out
